# revision 7
# baseline (speedup 1.0000x reference)
"""LocationAttention Trainium2 kernel.

Full inputs in, full outputs out. Internally: data-parallel over batch B=64
across 8 NeuronCores (8 batches/core). memory [T,B,Ck] is streamed from HBM
exactly once per core; key_att needs mem^T per Ck-block, produced on-chip via
TensorE transposes (PSUM) + ACT/DVE copies; conv folded into a single matmul
via UC = Uw @ conv_w; context matmul reuses the still-resident natural tiles
with unnormalized sigmoid weights, normalized per batch.
"""
import os
import sys

for _p in ("/opt/trn_rl_repo", "/root/.axon_site/_ro/trn_rl_repo"):
    if os.path.isdir(_p) and _p not in sys.path:
        sys.path.insert(0, _p)

import numpy as np

import concourse.bacc as bacc
import concourse.bass as bass
import concourse.mybir as mybir
import concourse.tile as tile
from concourse.bass_utils import run_bass_kernel_spmd
from concourse.masks import make_identity

T, B, Ck, Cq, A, F, KW = 2000, 64, 512, 1024, 128, 32, 31
NCORES = 8
BL = B // NCORES  # batches per core
PAD = KW // 2  # 15
TP = T + 2 * PAD + 2  # padded state row length (2032)
FP = mybir.dt.float32

NT = 16  # T tiles of 128 (last = 80)
TSZ = [128] * 15 + [80]
NCH = 4  # chunks of 4 tiles
CH0 = [0, 512, 1024, 1536]
CHN = [512, 512, 512, 464]

AF = mybir.ActivationFunctionType


def build_program():
    nc = bacc.Bacc(None, target_bir_lowering=False)

    mem_d = nc.dram_tensor("memory", [T, BL, Ck], FP, kind="ExternalInput")
    statep_d = nc.dram_tensor("statep", [BL, TP], FP, kind="ExternalInput")
    queryt_d = nc.dram_tensor("queryt", [Cq, BL], FP, kind="ExternalInput")
    wwt_big_d = nc.dram_tensor("Wwt", [Cq, A], FP, kind="ExternalInput")
    wbt_d = nc.dram_tensor("Wbt", [A, 1], FP, kind="ExternalInput")
    vwt_d = nc.dram_tensor("Vwt", [Ck, A], FP, kind="ExternalInput")
    uct_d = nc.dram_tensor("UCt", [KW, A], FP, kind="ExternalInput")
    wwt_d = nc.dram_tensor("wwt", [A, 1], FP, kind="ExternalInput")

    ctx_d = nc.dram_tensor("ctx", [BL, Ck], FP, kind="ExternalOutput")
    prob_d = nc.dram_tensor("prob", [BL, T], FP, kind="ExternalOutput")
    next_d = nc.dram_tensor("nextst", [BL, T], FP, kind="ExternalOutput")

    with tile.TileContext(nc) as tc:
        with (
            tc.tile_pool(name="const", bufs=1) as const,
            tc.tile_pool(name="mem", bufs=4) as memp,
            tc.tile_pool(name="memT", bufs=2) as memtp,
            tc.tile_pool(name="tanh", bufs=2) as tanhp,
            tc.tile_pool(name="shift", bufs=2) as shiftp,
            tc.tile_pool(name="srow", bufs=2) as srowp,
            tc.tile_pool(name="crow", bufs=2) as crowp,
            tc.tile_pool(name="ptr", bufs=2, space="PSUM") as ptrp,
            tc.tile_pool(name="pe", bufs=2, space="PSUM") as pep,
            tc.tile_pool(name="pe1", bufs=2, space="PSUM") as pe1p,
            tc.tile_pool(name="pctx", bufs=2, space="PSUM") as pctxp,
            tc.tile_pool(name="dram", bufs=1, space="DRAM") as dramp,
        ):
            # ---- constants / setup ----
            ident = const.tile([128, 128], FP)
            make_identity(nc, ident)

            vw_sb = const.tile([128, 4, A], FP)  # [p, q, a] = VwT[q*128+p, a]
            nc.scalar.dma_start(
                out=vw_sb, in_=vwt_d.rearrange("(q p) a -> p q a", p=128)
            )
            uc_sb = const.tile([KW, A], FP)
            nc.scalar.dma_start(out=uc_sb, in_=uct_d[:, :])
            ww_sb = const.tile([A, 1], FP)
            nc.scalar.dma_start(out=ww_sb, in_=wwt_d[:, :])
            wb_sb = const.tile([A, 1], FP)
            nc.scalar.dma_start(out=wb_sb, in_=wbt_d[:, :])

            # q_att = (query @ Ww.T).T + Wb'  -> qT_sb [A, BL]
            qin = const.tile([128, Cq // 128, BL], FP)
            nc.scalar.dma_start(
                out=qin, in_=queryt_d.rearrange("(q p) b -> p q b", p=128)
            )
            wwt_sb = const.tile([128, Cq // 128, A], FP)
            nc.scalar.dma_start(
                out=wwt_sb, in_=wwt_big_d.rearrange("(q p) a -> p q a", p=128)
            )
            q_ps = pep.tile([BL, A], FP, tag="pe_t")
            nq = Cq // 128
            for qc in range(nq):
                nc.tensor.matmul(
                    q_ps,
                    lhsT=qin[:, qc, :],
                    rhs=wwt_sb[:, qc, :],
                    start=(qc == 0),
                    stop=(qc == nq - 1),
                    skip_group_check=True,
                )
            q_sb = const.tile([BL, A], FP)
            nc.scalar.copy(q_sb, q_ps)
            qT_ps = ptrp.tile([A, BL], FP, tag="ptr_t")
            nc.tensor.transpose(qT_ps, q_sb, ident[0:BL, 0:BL])
            qT_sb = const.tile([A, BL], FP)
            nc.scalar.activation(qT_sb, qT_ps, AF.Identity, bias=wb_sb[:, 0:1])

            state_sb = const.tile([BL, T], FP)
            nc.scalar.dma_start(out=state_sb, in_=statep_d[:, PAD : PAD + T])

            ssum_all = const.tile([1, BL * NCH], FP)
            sT_all = const.tile([128, BL, NT], FP)
            s_dram = dramp.tile([BL, T], FP)

            # ---- main loop over local batches ----
            for b in range(BL):
                mem_half = []
                for h in range(2):
                    mh = memp.tile([128, 8, Ck], FP, tag="mem_h")
                    if h == 0:
                        nc.sync.dma_start(
                            out=mh,
                            in_=mem_d[0:1024, b, :].rearrange(
                                "(j p) c -> p j c", p=128
                            ),
                        )
                    else:
                        nc.sync.dma_start(
                            out=mh[:, 0:7, :],
                            in_=mem_d[1024:1920, b, :].rearrange(
                                "(j p) c -> p j c", p=128
                            ),
                        )
                        nc.sync.dma_start(
                            out=mh[0:80, 7, :], in_=mem_d[1920:2000, b, :]
                        )
                    mem_half.append(mh)

                shifted = shiftp.tile([KW, T], FP, tag="shift_t")
                nc.scalar.dma_start(
                    out=shifted,
                    in_=bass.AP(
                        tensor=statep_d, offset=b * TP, ap=[[1, KW], [1, T]]
                    ),
                )

                s_row = srowp.tile([1, T], FP, tag="s_row")
                for c in range(NCH):
                    half = mem_half[c // 2]
                    jb = (c % 2) * 4
                    n = CHN[c]
                    memT = memtp.tile([128, 4, 512], FP, tag="memT_t")
                    for j in range(4):
                        gi = 4 * c + j
                        tsz = TSZ[gi]
                        ptr_t = ptrp.tile([128, 512], FP, tag="ptr_t")
                        for q in range(4):
                            nc.tensor.transpose(
                                ptr_t[:, q * 128 : q * 128 + tsz],
                                half[0:tsz, jb + j, q * 128 : (q + 1) * 128],
                                ident[0:tsz, 0:tsz],
                            )
                        src = ptr_t.rearrange("p (q t) -> p q t", q=4)[:, :, 0:tsz]
                        dst = memT[:, :, j * 128 : j * 128 + tsz]
                        if j % 2 == 0:
                            nc.scalar.copy(dst, src)
                        else:
                            nc.vector.tensor_copy(dst, src)

                    pe_t = pep.tile([128, 512], FP, tag="pe_t")
                    for q in range(4):
                        nc.tensor.matmul(
                            pe_t[:, 0:n],
                            lhsT=vw_sb[:, q, :],
                            rhs=memT[:, q, 0:n],
                            start=(q == 0),
                            stop=False,
                            skip_group_check=True,
                        )
                    nc.tensor.matmul(
                        pe_t[:, 0:n],
                        lhsT=uc_sb,
                        rhs=shifted[:, CH0[c] : CH0[c] + n],
                        start=False,
                        stop=True,
                        skip_group_check=True,
                    )
                    tanh_t = tanhp.tile([128, 512], FP, tag="tanh_t")
                    nc.scalar.activation(
                        tanh_t[:, 0:n], pe_t[:, 0:n], AF.Tanh,
                        bias=qT_sb[:, b : b + 1],
                    )
                    pe1_t = pe1p.tile([1, 512], FP, tag="pe1_t")
                    nc.tensor.matmul(
                        pe1_t[0:1, 0:n],
                        lhsT=ww_sb,
                        rhs=tanh_t[:, 0:n],
                        start=True,
                        stop=True,
                        skip_group_check=True,
                    )
                    nc.scalar.activation(
                        s_row[0:1, CH0[c] : CH0[c] + n],
                        pe1_t[0:1, 0:n],
                        AF.Sigmoid,
                        accum_out=ssum_all[0:1, b * NCH + c : b * NCH + c + 1],
                    )

                # s -> sT via DRAM bounce (flat APs on both DMA legs)
                nc.scalar.dma_start(out=s_dram[b : b + 1, :], in_=s_row)
                sT_b = sT_all[:, b, :]
                nc.scalar.dma_start(
                    out=sT_b[:, 0:15],
                    in_=bass.AP(
                        tensor=s_dram.tensor,
                        offset=s_dram.offset + b * T,
                        ap=[[1, 128], [128, 15]],
                    ),
                )
                nc.scalar.dma_start(
                    out=sT_b[0:80, 15:16],
                    in_=bass.AP(
                        tensor=s_dram.tensor,
                        offset=s_dram.offset + b * T + 1920,
                        ap=[[1, 80], [128, 1]],
                    ),
                )

                pctx_t = pctxp.tile([1, Ck], FP, tag="pctx_t")
                for i in range(NT):
                    half = mem_half[i // 8]
                    j = i % 8
                    tsz = TSZ[i]
                    nc.tensor.matmul(
                        pctx_t[0:1, :],
                        lhsT=sT_b[0:tsz, i : i + 1],
                        rhs=half[0:tsz, j, :],
                        start=(i == 0),
                        stop=(i == NT - 1),
                        skip_group_check=True,
                    )
                # normalize ctx by 1/sum(s) for this batch (all on partition 0)
                sb1 = crowp.tile([1, 1], FP, tag="sb1")
                nc.vector.reduce_sum(
                    sb1,
                    ssum_all[0:1, b * NCH : (b + 1) * NCH],
                    axis=mybir.AxisListType.X,
                )
                rb1 = crowp.tile([1, 1], FP, tag="rb1")
                nc.vector.reciprocal(rb1, sb1)
                ctx_row = crowp.tile([1, Ck], FP, tag="ctx_row")
                nc.vector.tensor_scalar_mul(ctx_row, pctx_t[0:1, :], rb1)
                nc.scalar.dma_start(out=ctx_d[b : b + 1, :], in_=ctx_row)
                # stash 1/S for the prob/next_state finish
                nc.vector.tensor_copy(
                    ssum_all[0:1, b * NCH : b * NCH + 1], rb1
                )

            # ---- finish: prob = s/S, next = prob + state, in [8, T] layout ----
            rs_row = const.tile([1, BL], FP)
            nc.vector.tensor_copy(
                rs_row,
                ssum_all.rearrange("o (b c) -> o b c", b=BL)[:, :, 0],
            )
            rs_dram = dramp.tile([BL], FP)
            nc.scalar.dma_start(out=rs_dram[:], in_=rs_row[0:1, :])
            rS = const.tile([BL, 1], FP)
            nc.scalar.dma_start(
                out=rS, in_=rs_dram.rearrange("(b o) -> b o", o=1)
            )
            s_all = const.tile([BL, T], FP)
            nc.sync.dma_start(out=s_all, in_=s_dram[:, :])
            prob_sb = const.tile([BL, T], FP)
            nc.vector.tensor_scalar_mul(prob_sb, s_all, rS)
            next_sb = const.tile([BL, T], FP)
            nc.vector.tensor_add(next_sb, prob_sb, state_sb)
            nc.sync.dma_start(out=prob_d[:, :], in_=prob_sb)
            nc.sync.dma_start(out=next_d[:, :], in_=next_sb)

    nc.compile()
    return nc


_NC = None


def _get_nc():
    global _NC
    if _NC is None:
        _NC = build_program()
    return _NC


def kernel(memory, query, state, mask, Ww, Wb, Vw, Uw, conv_w, conv_b, ww,
           _want_trace=False):
    memory = np.ascontiguousarray(np.asarray(memory, dtype=np.float32))
    query = np.asarray(query, dtype=np.float32)
    state = np.asarray(state, dtype=np.float32)
    Ww = np.asarray(Ww, dtype=np.float32)
    Wb = np.asarray(Wb, dtype=np.float32)
    Vw = np.asarray(Vw, dtype=np.float32)
    Uw = np.asarray(Uw, dtype=np.float32)
    conv_w = np.asarray(conv_w, dtype=np.float32)
    conv_b = np.asarray(conv_b, dtype=np.float32)
    ww = np.asarray(ww, dtype=np.float32)

    # host-side weight prep (tiny)
    UC = Uw @ conv_w[:, 0, :]  # [A, KW]
    Wbp = Wb + Uw @ conv_b  # fold conv bias through Uw into the tanh bias
    vwt = np.ascontiguousarray(Vw.T)  # [Ck, A]
    uct = np.ascontiguousarray(UC.T)  # [KW, A]
    wwt_big = np.ascontiguousarray(Ww.T)  # [Cq, A]
    wwt = np.ascontiguousarray(ww.reshape(1, A).T)  # [A, 1]
    wbt = np.ascontiguousarray(Wbp.reshape(A, 1))

    in_maps = []
    for i in range(NCORES):
        bs = slice(i * BL, (i + 1) * BL)
        statep = np.zeros((BL, TP), dtype=np.float32)
        statep[:, PAD : PAD + T] = state[bs, 0, :]
        in_maps.append(
            {
                "memory": np.ascontiguousarray(memory[:, bs, :]),
                "statep": statep,
                "queryt": np.ascontiguousarray(query[0, bs, :].T),
                "Wwt": wwt_big,
                "Wbt": wbt,
                "Vwt": vwt,
                "UCt": uct,
                "wwt": wwt,
            }
        )

    nc = _get_nc()
    res = run_bass_kernel_spmd(
        nc, in_maps, core_ids=list(range(NCORES)), trace=_want_trace
    )

    context = np.empty((1, B, Ck), dtype=np.float32)
    prob = np.empty((B, 1, T), dtype=np.float32)
    next_state = np.empty((B, 1, T), dtype=np.float32)
    for i in range(NCORES):
        bs = slice(i * BL, (i + 1) * BL)
        r = res.results[i]
        context[0, bs, :] = r["ctx"]
        prob[bs, 0, :] = r["prob"]
        next_state[bs, 0, :] = r["nextst"]
    if _want_trace:
        return (context, prob, next_state), res
    return context, prob, next_state
